# revision 1
# baseline (speedup 1.0000x reference)
"""Multi-level (FPN) DeformRoIPool (zero-offset == aligned RoIAlign) for Trainium2.

Strategy (8 NeuronCores, SPMD, one Bass program):
- Shard the 256 ROIs across cores (32 each); feature maps are preprocessed on
  host into per-ROI gather windows (channels-last pair-rows), so each core only
  uploads/reads the rows its ROIs touch.
- Per sample point (7x7 bins x 2x2 samples = 196 per ROI) one dma_gather
  element of 4KB covers the whole 2x2 bilinear patch: the window stores row
  pairs [F(y), F(y+1 clamped)] per (y, x) position (512 f32), and the gather
  element spans two consecutive x positions (1024 f32, overlapping stride).
- The weighted reduction over (sample, corner) -> (bin) runs on the PE as
  small matmuls with a host-built sparse weight matrix per ROI, accumulating
  in PSUM [49 bins, 256 c]. Host transposes [roi, bin, c] -> [roi, c, 7, 7].
"""
import os
import sys
import types

import numpy as np

OUT = 7
SR = 2
STRIDES = (4, 8, 16, 32)
FINEST = 56.0
IMG = 800.0
NLEV = 4
C = 256
N_ROIS = 256
N_CORES = 8
NROI_C = N_ROIS // N_CORES          # 32 rois per core
ROIS_PER_CALL = 4
NCALL = NROI_C // ROIS_PER_CALL     # gather calls per core
NSAMP = OUT * OUT * SR * SR         # 196 samples per roi
NREAL_CALL = ROIS_PER_CALL * NSAMP  # real gather idxs per call
NI_CALL = -(-NREAL_CALL // 16) * 16  # padded to x16 with trailing -1 (skipped)
NGRP_CALL = -(-NREAL_CALL // 128)   # slot groups per call
# flat (group, roi) matmul sets
GROUP_SETS = []
GROUP_K = []
for _g in range(NGRP_CALL):
    _lo, _hi = _g * 128, min((_g + 1) * 128, NREAL_CALL)
    GROUP_K.append(_hi - _lo)
    for _j in range(_lo // NSAMP, (_hi - 1) // NSAMP + 1):
        GROUP_SETS.append((_g, _j))
NSETS = len(GROUP_SETS)
WIN_R = 14 * 200                    # pair-row positions reserved per roi (l0 worst case)
WIN_STRIDE = WIN_R + 1              # +1 guard row per roi block
FEAT_SHAPES = [(2, 256, 200, 200), (2, 256, 100, 100), (2, 256, 50, 50), (2, 256, 25, 25)]


# ---------------------------------------------------------------------------
# BIR fix: this container's walrus rejects >1 embedded sem wait per
# instruction (2 on EventSemaphore). Split excess waits onto EventSemaphore
# carriers at serialization time.
# ---------------------------------------------------------------------------
def _install_bir_waitsplit():
    import orjson
    import concourse.bass as bass

    if getattr(bass.Bass, "_waitsplit_patched", False):
        return

    def _fix_blocks(blocks, counter):
        for blk in blocks:
            insts = blk.get("instructions")
            if insts:
                out = []
                for ins in insts:
                    si = ins.get("sync_info")
                    ow = (si or {}).get("on_wait") or []
                    limit = 2 if ins.get("opcode") == "EventSemaphore" else 1
                    if len(ow) > limit:
                        excess = ow[: len(ow) - limit]
                        si["on_wait"] = ow[len(ow) - limit:]
                        for i in range(0, len(excess), 2):
                            counter[0] += 1
                            out.append({
                                "name": f"I-waitsplit-{counter[0]}",
                                "opcode": "EventSemaphore",
                                "engine": ins["engine"],
                                "ins": [], "outs": [],
                                "debug": ins.get("debug", 0),
                                "sync_info": {"on_update": [], "on_wait": excess[i:i + 2]},
                            })
                    out.append(ins)
                blk["instructions"] = out
            if blk.get("blocks"):
                _fix_blocks(blk["blocks"], counter)

    orig = bass.Bass.to_json_bytes

    def to_json_bytes(self, *a, **kw):
        data = orig(self, *a, **kw)
        d = orjson.loads(data)
        counter = [0]
        for fn in d.get("functions", []):
            _fix_blocks(fn.get("blocks", []), counter)
        return orjson.dumps(d) if counter[0] else data

    bass.Bass.to_json_bytes = to_json_bytes
    bass.Bass._waitsplit_patched = True


# ---------------------------------------------------------------------------
# Host-side index / weight / window computation
# ---------------------------------------------------------------------------
def _roi_meta(rois):
    """Per-roi level + sample-grid floors and weights.

    Returns list of dicts with level l, batch b, and per-(i,si)/(j,sj) arrays.
    """
    scale_wh = np.sqrt((rois[:, 3] - rois[:, 1]) * (rois[:, 4] - rois[:, 2]))
    with np.errstate(divide="ignore"):
        tl = np.clip(np.floor(np.log2(scale_wh / FINEST + 1e-6)), 0, NLEV - 1)
    tl = (tl + 1e-5).astype(np.int32)
    g = np.arange(OUT, dtype=np.float64)[:, None] + (np.arange(SR, dtype=np.float64)[None, :] + 0.5) / SR
    metas = []
    for n in range(rois.shape[0]):
        l = int(tl[n])
        B, C_, H, W = FEAT_SHAPES[l]
        sc = 1.0 / STRIDES[l]
        x1 = rois[n, 1] * sc - 0.5
        y1 = rois[n, 2] * sc - 0.5
        rw = rois[n, 3] * sc - 0.5 - x1
        rh = rois[n, 4] * sc - 0.5 - y1
        y = y1 + (rh / OUT) * g  # [OUT, SR] sample y per (i, si)
        x = x1 + (rw / OUT) * g
        vy = (y > -1) & (y < H)
        vx = (x > -1) & (x < W)
        yc = np.clip(y, 0.0, H - 1)
        xc = np.clip(x, 0.0, W - 1)
        y0 = np.minimum(np.floor(yc).astype(np.int64), H - 1)
        x0 = np.minimum(np.floor(xc).astype(np.int64), W - 1)
        metas.append(dict(
            l=l, b=int(rois[n, 0]), H=H, W=W,
            y0=y0, x0=x0, ly=yc - y0, lx=xc - x0, vy=vy, vx=vx,
        ))
    return metas


def _build_core_inputs(feats_T, metas, core_rois):
    """Build win/idx/W tensors for one core's roi list (indices into metas)."""
    win = np.zeros((NROI_C * WIN_STRIDE + 1, 2 * C), np.float32)
    idx_all = np.full((NCALL, NI_CALL), -1, np.int16)
    wmat = np.zeros((NCALL, NSETS, 128, 4 * 49), np.float32)
    set_of = {(g_, j_): si_ for si_, (g_, j_) in enumerate(GROUP_SETS)}

    for rslot, n in enumerate(core_rois):
        m = metas[n]
        H, W = m["H"], m["W"]
        fT = feats_T[m["l"]][m["b"]]  # [H, W, C] channels-last view
        ys, yrank_of = np.unique(m["y0"]), {}
        for k, yv in enumerate(ys):
            yrank_of[yv] = k
        ysp1 = np.minimum(ys + 1, H - 1)
        nY = len(ys)
        # window block: rows [k*W + x] = [F(ys[k], x, :) | F(ys[k]+1c, x, :)]
        base = rslot * WIN_STRIDE
        blk = win[base:base + nY * W].reshape(nY, W, 2 * C)
        blk[:, :, :C] = fT[ys]
        blk[:, :, C:] = fT[ysp1]

        call, j = rslot // ROIS_PER_CALL, rslot % ROIS_PER_CALL
        jbase = j * WIN_STRIDE  # idx base within the call's 4-roi window span
        y0, x0, ly, lx = m["y0"], m["x0"], m["ly"], m["lx"]
        vy, vx = m["vy"], m["vx"]
        for i in range(OUT):
            for jj in range(OUT):
                for si in range(SR):
                    for sj in range(SR):
                        s = ((i * OUT + jj) * 4) + si * 2 + sj
                        slot = j * NSAMP + s
                        g_, p_ = slot // 128, slot % 128
                        yy0 = y0[i, si]
                        xx0 = x0[jj, sj]
                        idx_all[call, slot] = jbase + yrank_of[yy0] * W + xx0
                        v = (vy[i, si] and vx[jj, sj]) / (SR * SR)
                        hy = (1.0 - ly[i, si]) * v
                        lyv = ly[i, si] * v
                        hx = 1.0 - lx[jj, sj]
                        lxv = lx[jj, sj]
                        w0, w1, w2, w3 = hy * hx, lyv * hx, hy * lxv, lyv * lxv
                        if xx0 == W - 1:  # x1 clamps onto x0
                            w0, w2 = w0 + w2, 0.0
                            w1, w3 = w1 + w3, 0.0
                        b = s // 4
                        si_ = set_of[(g_, j)]
                        for q, w in enumerate((w0, w1, w2, w3)):
                            wmat[call, si_, p_, q * 49 + b] = w

    # idx layout per call: [128, NI/16], slot i -> [i%16, i//16], replicated x8
    idx_tiles = np.zeros((128, NCALL * (NI_CALL // 16)), np.int16)
    for c in range(NCALL):
        blk16 = idx_all[c].reshape(NI_CALL // 16, 16).T
        idx_tiles[:, c * (NI_CALL // 16):(c + 1) * (NI_CALL // 16)] = np.tile(blk16, (8, 1))
    return win, idx_tiles, wmat


def _build_core_inputs_fp16(feats_T, metas, core_rois):
    win, idx_tiles, wmat = _build_core_inputs(feats_T, metas, core_rois)
    return win.astype(np.float16), idx_tiles, wmat.astype(np.float16)


def _build_program():
    import concourse.bacc as bacc
    import concourse.mybir as mybir
    import concourse.tile as tile

    _install_bir_waitsplit()
    nc = bacc.Bacc("TRN2", debug=False, enable_asserts=True, num_devices=N_CORES)
    import concourse.bass as bass

    win_rows = NROI_C * WIN_STRIDE + 1
    win_d = nc.dram_tensor("win", [win_rows, 2 * C], mybir.dt.float16, kind="ExternalInput")
    idx_d = nc.dram_tensor("idx", [128, NCALL * (NI_CALL // 16)], mybir.dt.int16, kind="ExternalInput")
    w_d = nc.dram_tensor("wts", [NCALL * NSETS, 128, 4 * 49], mybir.dt.float16, kind="ExternalInput")
    out_d = nc.dram_tensor("out", [NROI_C, 49 * C], mybir.dt.float16, kind="ExternalOutput")


    with tile.TileContext(nc) as tc:
        with (
            tc.tile_pool(name="ip", bufs=1) as ip,
            tc.tile_pool(name="gp", bufs=8) as gp,
            tc.tile_pool(name="sp", bufs=3) as sp,
            tc.tile_pool(name="pp", bufs=8, space="PSUM") as pp,
        ):
            idx_t = ip.tile([128, NCALL * (NI_CALL // 16)], mybir.dt.int16)
            nc.sync.dma_start(idx_t[:], idx_d[:])
            wt = ip.tile([128, NCALL * NSETS * 4 * 49], mybir.dt.float16)
            nc.sync.dma_start(
                wt[:].rearrange("p (r w) -> p r w", w=4 * 49),
                w_d[:].rearrange("r p w -> p r w"),
            )
            for call in range(NCALL):
                g = gp.tile([128, NGRP_CALL * 4 * C], mybir.dt.float16, tag="g")
                # overlapping 4KB elems: row step 512 f32, elem 1024 f32
                src = bass.AP(
                    win_d[:].tensor,
                    call * ROIS_PER_CALL * WIN_STRIDE * (2 * C),
                    [[2 * C, ROIS_PER_CALL * WIN_STRIDE], [1, 4 * C]],
                )
                nc.gpsimd.dma_gather(
                    out_ap=g[:].rearrange("p (k c) -> p k c", c=4 * C),
                    in_ap=src,
                    idxs_ap=idx_t[:, call * (NI_CALL // 16):(call + 1) * (NI_CALL // 16)],
                    num_idxs=NI_CALL,
                    num_idxs_reg=NREAL_CALL,
                    elem_size=4 * C,
                    elem_step=2 * C,
                    single_packet=False,
                )
                st = sp.tile([49, ROIS_PER_CALL * C], mybir.dt.float16, tag="st")
                # first/last set index per roi j for start/stop flags
                firsts, lasts = {}, {}
                for si_, (g_, j_) in enumerate(GROUP_SETS):
                    firsts.setdefault(j_, si_)
                    lasts[j_] = si_
                ps_of = {j_: pp.tile([49, C], mybir.dt.float32, tag="ps", name=f"ps_{call}_{j_}") for j_ in range(ROIS_PER_CALL)}
                for si_, (g_, j_) in enumerate(GROUP_SETS):
                    K = GROUP_K[g_]
                    ps = ps_of[j_]
                    wb = (call * NSETS + si_) * 4 * 49
                    for q in range(4):
                        nc.tensor.matmul(
                            out=ps[:, :],
                            lhsT=wt[0:K, wb + q * 49:wb + (q + 1) * 49],
                            rhs=g[0:K, g_ * 4 * C + q * C:g_ * 4 * C + (q + 1) * C],
                            start=(si_ == firsts[j_] and q == 0),
                            stop=(si_ == lasts[j_] and q == 3),
                        )
                for j_ in range(ROIS_PER_CALL):
                    nc.vector.tensor_copy(st[:, j_ * C:(j_ + 1) * C], ps_of[j_][:])
                nc.sync.dma_start(
                    out_d[call * ROIS_PER_CALL:(call + 1) * ROIS_PER_CALL].rearrange(
                        "r (b c) -> b r c", c=C
                    ),
                    st[:].rearrange("b (r c) -> b r c", c=C),
                )
    nc.compile()
    return nc


def kernel(feat0, feat1, feat2, feat3, rois):
    from concourse.bass_utils import run_bass_kernel_spmd

    feats = [np.asarray(f, np.float32) for f in (feat0, feat1, feat2, feat3)]
    rois = np.asarray(rois, np.float32)
    # channels-last views per level/batch
    feats_T = [np.ascontiguousarray(f.transpose(0, 2, 3, 1)) for f in feats]
    metas = _roi_meta(rois)

    in_maps = []
    for core in range(N_CORES):
        core_rois = list(range(core * NROI_C, (core + 1) * NROI_C))
        win, idx_tiles, wmat = _build_core_inputs_fp16(feats_T, metas, core_rois)
        in_maps.append({"win": win, "idx": idx_tiles, "wts": wmat.reshape(NCALL * NSETS, 128, 4 * 49)})

    nc = _build_program()
    res = run_bass_kernel_spmd(nc, in_maps, core_ids=list(range(N_CORES)), trace=False)
    outs = []
    for core in range(N_CORES):
        o = res.results[core]["out"].astype(np.float32).reshape(NROI_C, 49, C)
        outs.append(np.ascontiguousarray(o.transpose(0, 2, 1)).reshape(NROI_C, C, OUT, OUT))
    return np.concatenate(outs, 0)


# Testing hook: emulate the device math in numpy (same win/idx/W data).
def emulate(feat0, feat1, feat2, feat3, rois):
    feats = [np.asarray(f, np.float32) for f in (feat0, feat1, feat2, feat3)]
    rois = np.asarray(rois, np.float32)
    feats_T = [np.ascontiguousarray(f.transpose(0, 2, 3, 1)) for f in feats]
    metas = _roi_meta(rois)
    out = np.zeros((N_ROIS, C, OUT, OUT), np.float32)
    for core in range(N_CORES):
        core_rois = list(range(core * NROI_C, (core + 1) * NROI_C))
        win, idx_tiles, wmat = _build_core_inputs(feats_T, metas, core_rois)
        winf = win.reshape(-1)
        for call in range(NCALL):
            idx_blk = idx_tiles[:16, call * (NI_CALL // 16):(call + 1) * (NI_CALL // 16)]
            slots = idx_blk.T.reshape(-1)
            base_off = call * ROIS_PER_CALL * WIN_STRIDE * (2 * C)
            G = np.zeros((NI_CALL, 4 * C), np.float32)
            for i in range(NREAL_CALL):
                st = base_off + int(slots[i]) * 2 * C
                G[i] = winf[st:st + 4 * C]
            accs = [np.zeros((49, C), np.float32) for _ in range(ROIS_PER_CALL)]
            for si_, (g_, j_) in enumerate(GROUP_SETS):
                K = GROUP_K[g_]
                W_ = wmat[call, si_]
                for q in range(4):
                    accs[j_] += W_[0:K, q * 49:(q + 1) * 49].T @ G[g_ * 128:g_ * 128 + K, q * C:(q + 1) * C]
            for j_ in range(ROIS_PER_CALL):
                r = core_rois[call * ROIS_PER_CALL + j_]
                out[r] = accs[j_].T.reshape(C, OUT, OUT)
    return out



# revision 4
# speedup vs baseline: 1.9876x; 1.9876x over previous
"""Multi-level (FPN) DeformRoIPool (zero-offset == aligned RoIAlign) for Trainium2.

Strategy (8 NeuronCores, SPMD, one Bass program):
- Host computes, per ROI, the unique corner positions (y, x) its 196 bilinear
  samples touch and packs those feature rows (C=256, fp16) into a contiguous
  per-core stream; duplicate corners are fetched once (~2.7x dedup vs per-
  sample gathers). ROIs are LPT-balanced across cores into 32 fixed "slots"
  with per-slot row capacities shared by all cores, so one compiled program
  serves all 8 cores.
- Device does plain chunked dma_starts (no gather): stream chunks + matching
  weight blocks double/triple-buffered, then one [K=128, 49] x [K=128, C]
  matmul per (group, slot) accumulating bins in PSUM (fp32). Two slots share
  one PSUM bank [49, 512]; DVE drains pairs to an fp16 staging tile; 4 batched
  stores write [98, 1024] blocks out.
- Host unpacks [slot, bin, C] -> [N, C, 7, 7].
"""
import numpy as np

OUT = 7
SR = 2
STRIDES = (4, 8, 16, 32)
FINEST = 56.0
NLEV = 4
C = 256
N_ROIS = 256
N_CORES = 8
NSLOT = N_ROIS // N_CORES           # 32 roi slots per core
NBIN = OUT * OUT                    # 49
CHUNK_G = 6                         # stream groups (of 128 rows) per DMA chunk
FEAT_SHAPES = [(2, 256, 200, 200), (2, 256, 100, 100), (2, 256, 50, 50), (2, 256, 25, 25)]


# ---------------------------------------------------------------------------
# BIR fix: this container's walrus rejects >1 embedded sem wait per
# instruction (2 on EventSemaphore). Split excess waits onto EventSemaphore
# carriers at serialization time.
# ---------------------------------------------------------------------------
def _install_bir_waitsplit():
    import orjson
    import concourse.bass as bass

    if getattr(bass.Bass, "_waitsplit_patched", False):
        return

    def _fix_blocks(blocks, counter):
        for blk in blocks:
            insts = blk.get("instructions")
            if insts:
                out = []
                for ins in insts:
                    si = ins.get("sync_info")
                    ow = (si or {}).get("on_wait") or []
                    limit = 2 if ins.get("opcode") == "EventSemaphore" else 1
                    if len(ow) > limit:
                        excess = ow[: len(ow) - limit]
                        si["on_wait"] = ow[len(ow) - limit:]
                        for i in range(0, len(excess), 2):
                            counter[0] += 1
                            out.append({
                                "name": f"I-waitsplit-{counter[0]}",
                                "opcode": "EventSemaphore",
                                "engine": ins["engine"],
                                "ins": [], "outs": [],
                                "debug": ins.get("debug", 0),
                                "sync_info": {"on_update": [], "on_wait": excess[i:i + 2]},
                            })
                    out.append(ins)
                blk["instructions"] = out
            if blk.get("blocks"):
                _fix_blocks(blk["blocks"], counter)

    orig = bass.Bass.to_json_bytes

    def to_json_bytes(self, *a, **kw):
        data = orig(self, *a, **kw)
        d = orjson.loads(data)
        counter = [0]
        for fn in d.get("functions", []):
            _fix_blocks(fn.get("blocks", []), counter)
        return orjson.dumps(d) if counter[0] else data

    bass.Bass.to_json_bytes = to_json_bytes
    bass.Bass._waitsplit_patched = True


# ---------------------------------------------------------------------------
# Host-side planning: per-roi corner grids, core/slot assignment, set layout
# ---------------------------------------------------------------------------
def _roi_meta(rois):
    """Per-roi level/batch + unique corner coords and per-sample corner ranks
    and bilinear weights."""
    scale_wh = np.sqrt((rois[:, 3] - rois[:, 1]) * (rois[:, 4] - rois[:, 2]))
    with np.errstate(divide="ignore"):
        tl = np.clip(np.floor(np.log2(scale_wh / FINEST + 1e-6)), 0, NLEV - 1)
    tl = (tl + 1e-5).astype(np.int32)
    g = np.arange(OUT, dtype=np.float64)[:, None] + (np.arange(SR, dtype=np.float64)[None, :] + 0.5) / SR
    metas = []
    for n in range(rois.shape[0]):
        l = int(tl[n])
        B, C_, H, W = FEAT_SHAPES[l]
        sc = 1.0 / STRIDES[l]
        x1 = rois[n, 1] * sc - 0.5
        y1 = rois[n, 2] * sc - 0.5
        rw = rois[n, 3] * sc - 0.5 - x1
        rh = rois[n, 4] * sc - 0.5 - y1
        y = y1 + (rh / OUT) * g  # [OUT, SR]
        x = x1 + (rw / OUT) * g
        vy = (y > -1) & (y < H)
        vx = (x > -1) & (x < W)
        yc = np.clip(y, 0.0, H - 1)
        xc = np.clip(x, 0.0, W - 1)
        y0 = np.minimum(np.floor(yc).astype(np.int64), H - 1)
        x0 = np.minimum(np.floor(xc).astype(np.int64), W - 1)
        y1i = np.minimum(y0 + 1, H - 1)
        x1i = np.minimum(x0 + 1, W - 1)
        ly = (yc - y0) * vy
        lx = (xc - x0) * vx
        hy = (1.0 - (yc - y0)) * vy
        hx = (1.0 - (xc - x0)) * vx
        ys = np.unique(np.concatenate([y0.ravel(), y1i.ravel()]))
        xs = np.unique(np.concatenate([x0.ravel(), x1i.ravel()]))
        metas.append(dict(
            l=l, b=int(rois[n, 0]), ys=ys, xs=xs, nrows=len(ys) * len(xs),
            y0r=np.searchsorted(ys, y0), y1r=np.searchsorted(ys, y1i),
            x0r=np.searchsorted(xs, x0), x1r=np.searchsorted(xs, x1i),
            ly=ly, lx=lx, hy=hy, hx=hx,
        ))
    return metas


def _plan(metas):
    """Assign rois to (core, slot); fixed slot row-caps shared by all cores."""
    nrows = np.array([m["nrows"] for m in metas])
    order = np.argsort(-nrows, kind="stable")
    loads = np.zeros(N_CORES, np.int64)
    counts = np.zeros(N_CORES, np.int64)
    core_rois = [[] for _ in range(N_CORES)]
    for i in order:
        avail = np.where(counts < NSLOT)[0]
        c = avail[np.argmin(loads[avail])]
        core_rois[c].append(int(i))
        loads[c] += nrows[i]
        counts[c] += 1
    # each core's list is already sorted desc by rows (dealt in global order)
    caps = np.zeros(NSLOT, np.int64)
    for c in range(N_CORES):
        for j, r in enumerate(core_rois[c]):
            caps[j] = max(caps[j], nrows[r])
    off = np.zeros(NSLOT + 1, np.int64)
    off[1:] = np.cumsum(caps)
    T = int(off[-1])
    G = -(-T // 128)
    # sets: one matmul per (group, slot) pair, ordered by (g, j)
    per_g = [[] for _ in range(G)]
    for j in range(NSLOT):
        g0, g1 = off[j] // 128, (off[j + 1] - 1) // 128
        for g in range(g0, g1 + 1):
            per_g[g].append((j, g == g0, g == g1))
    sets = []
    setmap = {}
    for g in range(G):
        for (j, st, sp) in per_g[g]:
            setmap[(g, j)] = len(sets)
            sets.append((g, j, st, sp))
    return core_rois, off, G, sets, setmap


def _build_core_inputs(feats16, metas, core_list, off, G, sets, setmap):
    S = len(sets)
    stream = np.zeros((128, G, C), np.float16)
    wacc = np.zeros((S, 128, NBIN), np.float32)
    for j, r in enumerate(core_list):
        m = metas[r]
        ys, xs = m["ys"], m["xs"]
        nX = len(xs)
        fT = feats16[m["l"]][m["b"]]  # [H, W, C] channels-last fp16
        block = fT[np.ix_(ys, xs)].reshape(-1, C)
        rows = int(off[j]) + np.arange(block.shape[0])
        stream[rows % 128, rows // 128, :] = block
        # bilinear weights onto (corner-row, bin)
        bi = (np.arange(OUT)[:, None, None, None] * OUT
              + np.arange(OUT)[None, None, :, None])  # [7,1,7,1]
        bins = np.broadcast_to(bi, (OUT, SR, OUT, SR))
        y0r, y1r = m["y0r"], m["y1r"]  # [7,2]
        x0r, x1r = m["x0r"], m["x1r"]
        ly, hy = m["ly"], m["hy"]
        lx, hx = m["lx"], m["hx"]
        q = 1.0 / (SR * SR)
        rows4, w4 = [], []
        for yr, wy in ((y0r, hy), (y1r, ly)):
            for xr, wx in ((x0r, hx), (x1r, lx)):
                rr = yr[:, :, None, None] * nX + xr[None, None, :, :]
                ww = wy[:, :, None, None] * wx[None, None, :, :] * q
                rows4.append(np.broadcast_to(rr, bins.shape).ravel())
                w4.append(np.broadcast_to(ww, bins.shape).ravel())
        rowa = int(off[j]) + np.concatenate(rows4)
        wa = np.concatenate(w4)
        bina = np.tile(bins.ravel(), 4)
        ga = rowa // 128
        sa = np.array([setmap[(int(g), j)] for g in ga])
        np.add.at(wacc, (sa, rowa % 128, bina), wa)
    wts = np.ascontiguousarray(wacc.astype(np.float16).transpose(1, 0, 2)).reshape(128, S * NBIN)
    return stream.reshape(128, G * C), wts


# ---------------------------------------------------------------------------
# Bass program
# ---------------------------------------------------------------------------
def _build_program(G, S, sets):
    import concourse.bacc as bacc
    import concourse.mybir as mybir
    import concourse.tile as tile

    _install_bir_waitsplit()
    nc = bacc.Bacc("TRN2", debug=False, enable_asserts=True, num_devices=N_CORES)

    stream_d = nc.dram_tensor("stream", [128, G * C], mybir.dt.float16, kind="ExternalInput")
    w_d = nc.dram_tensor("wts", [128, S * NBIN], mybir.dt.float16, kind="ExternalInput")
    out_d = nc.dram_tensor("out", [NBIN, NSLOT * C], mybir.dt.float16, kind="ExternalOutput")

    nchunk = -(-G // CHUNK_G)
    # sets are sorted by group -> contiguous set range per chunk
    chunk_sets = []
    for cidx in range(nchunk):
        glo, ghi = cidx * CHUNK_G, min((cidx + 1) * CHUNK_G, G)
        s_in = [s for s, (g, j, st, sp) in enumerate(sets) if glo <= g < ghi]
        chunk_sets.append((glo, ghi, s_in[0], s_in[-1] + 1))
    max_sets = max(hi - lo for _, _, lo, hi in chunk_sets)

    with tile.TileContext(nc) as tc:
        with (
            tc.tile_pool(name="fp", bufs=3) as fp,
            tc.tile_pool(name="wp", bufs=3) as wp,
            tc.tile_pool(name="sp", bufs=4) as sp,
            tc.tile_pool(name="pp", bufs=8, space="PSUM") as pp,
        ):
            st_tiles = [sp.tile([NBIN, 8 * C], mybir.dt.float16, tag=f"st{k}", name=f"st{k}")
                        for k in range(4)]
            ps_of = {}
            for cidx, (glo, ghi, slo, shi) in enumerate(chunk_sets):
                ft = fp.tile([128, CHUNK_G * C], mybir.dt.float16, tag="ft", name=f"ft{cidx}")
                nc.sync.dma_start(ft[:, :(ghi - glo) * C], stream_d[:, glo * C:ghi * C])
                wt = wp.tile([128, max_sets * NBIN], mybir.dt.float16, tag="wt", name=f"wt{cidx}")
                nc.sync.dma_start(wt[:, :(shi - slo) * NBIN], w_d[:, slo * NBIN:shi * NBIN])
                for s in range(slo, shi):
                    g, j, st, sp_ = sets[s]
                    t = j // 2
                    if st and j % 2 == 0:
                        ps_of[t] = pp.tile([NBIN, 2 * C], mybir.dt.float32, tag="ps", name=f"ps_{t}")
                    ps = ps_of[t]
                    half = (j % 2) * C
                    nc.tensor.matmul(
                        out=ps[:, half:half + C],
                        lhsT=wt[:, (s - slo) * NBIN:(s - slo + 1) * NBIN],
                        rhs=ft[:, (g - glo) * C:(g - glo + 1) * C],
                        start=st, stop=sp_,
                    )
                    if sp_ and j % 2 == 1:
                        k, q = t // 4, t % 4
                        dst = st_tiles[k][:, q * 2 * C:(q + 1) * 2 * C]
                        nc.vector.tensor_copy(dst, ps[:, :])
                        if q == 3:
                            nc.sync.dma_start(
                                out_d[:, k * 8 * C:(k + 1) * 8 * C], st_tiles[k][:])
    nc.compile()
    return nc


# ---------------------------------------------------------------------------
# Entry points
# ---------------------------------------------------------------------------
def _prepare(feat0, feat1, feat2, feat3, rois):
    feats = [np.asarray(f, np.float32) for f in (feat0, feat1, feat2, feat3)]
    rois = np.asarray(rois, np.float32)
    feats16 = [f.transpose(0, 2, 3, 1).astype(np.float16) for f in feats]
    metas = _roi_meta(rois)
    core_rois, off, G, sets, setmap = _plan(metas)
    in_maps = []
    for c in range(N_CORES):
        stream, wts = _build_core_inputs(feats16, metas, core_rois[c], off, G, sets, setmap)
        in_maps.append({"stream": stream, "wts": wts})
    return core_rois, G, sets, in_maps


def _unpack(out_np, core_list):
    """[49, NSLOT*C] fp16 device output -> dict roi_id -> [C, 7, 7] fp32."""
    res = {}
    for j in range(NSLOT):
        v = out_np[:, j * C:(j + 1) * C].astype(np.float32)  # [49, C]
        res[core_list[j]] = np.ascontiguousarray(v.T).reshape(C, OUT, OUT)
    return res


def kernel(feat0, feat1, feat2, feat3, rois):
    from concourse.bass_utils import run_bass_kernel_spmd

    core_rois, G, sets, in_maps = _prepare(feat0, feat1, feat2, feat3, rois)
    nc = _build_program(G, len(sets), sets)
    res = run_bass_kernel_spmd(nc, in_maps, core_ids=list(range(N_CORES)), trace=False)
    out = np.zeros((N_ROIS, C, OUT, OUT), np.float32)
    for c in range(N_CORES):
        for rid, v in _unpack(res.results[c]["out"], core_rois[c]).items():
            out[rid] = v
    return out


# Testing hook: emulate the device math in numpy (same stream/wts data).
def emulate(feat0, feat1, feat2, feat3, rois):
    core_rois, G, sets, in_maps = _prepare(feat0, feat1, feat2, feat3, rois)
    out = np.zeros((N_ROIS, C, OUT, OUT), np.float32)
    for c in range(N_CORES):
        stream = in_maps[c]["stream"].reshape(128, G, C)
        wts = in_maps[c]["wts"].reshape(128, len(sets), NBIN)
        acc = {}
        for s, (g, j, st, sp_) in enumerate(sets):
            part = wts[:, s, :].astype(np.float32).T @ stream[:, g, :].astype(np.float32)
            if st:
                acc[j] = part
            else:
                acc[j] += part
        dev = np.zeros((NBIN, NSLOT * C), np.float16)
        for j in range(NSLOT):
            dev[:, j * C:(j + 1) * C] = acc[j].astype(np.float16)
        for rid, v in _unpack(dev, core_rois[c]).items():
            out[rid] = v
    return out


# revision 5
# speedup vs baseline: 2.2343x; 1.1241x over previous
"""Multi-level (FPN) DeformRoIPool (zero-offset == aligned RoIAlign) for Trainium2.

Strategy (8 NeuronCores, SPMD, one Bass program):
- Host computes, per ROI, the unique corner positions (y, x) its 196 bilinear
  samples touch and packs those feature rows (C=256, fp16) into a contiguous
  per-core stream; duplicate corners are fetched once (~2.7x dedup vs per-
  sample gathers). ROIs are LPT-balanced across cores into 32 fixed "slots"
  with per-slot row capacities shared by all cores, so one compiled program
  serves all 8 cores.
- Device does plain chunked dma_starts (no gather): stream chunks + matching
  weight blocks double/triple-buffered, then one [K=128, 49] x [K=128, C]
  matmul per (group, slot) accumulating bins in PSUM (fp32). Two slots share
  one PSUM bank [49, 512]; DVE drains pairs to an fp16 staging tile; 4 batched
  stores write [98, 1024] blocks out.
- Host unpacks [slot, bin, C] -> [N, C, 7, 7].
"""
import numpy as np

OUT = 7
SR = 2
STRIDES = (4, 8, 16, 32)
FINEST = 56.0
NLEV = 4
C = 256
N_ROIS = 256
N_CORES = 8
NSLOT = N_ROIS // N_CORES           # 32 roi slots per core
NBIN = OUT * OUT                    # 49
CHUNK_G = 10                        # stream groups (of 128 rows) per DMA chunk
FEAT_SHAPES = [(2, 256, 200, 200), (2, 256, 100, 100), (2, 256, 50, 50), (2, 256, 25, 25)]


# ---------------------------------------------------------------------------
# BIR fix: this container's walrus rejects >1 embedded sem wait per
# instruction (2 on EventSemaphore). Split excess waits onto EventSemaphore
# carriers at serialization time.
# ---------------------------------------------------------------------------
def _install_bir_waitsplit():
    import orjson
    import concourse.bass as bass

    if getattr(bass.Bass, "_waitsplit_patched", False):
        return

    def _fix_blocks(blocks, counter):
        for blk in blocks:
            insts = blk.get("instructions")
            if insts:
                out = []
                for ins in insts:
                    si = ins.get("sync_info")
                    ow = (si or {}).get("on_wait") or []
                    limit = 2 if ins.get("opcode") == "EventSemaphore" else 1
                    if len(ow) > limit:
                        excess = ow[: len(ow) - limit]
                        si["on_wait"] = ow[len(ow) - limit:]
                        for i in range(0, len(excess), 2):
                            counter[0] += 1
                            out.append({
                                "name": f"I-waitsplit-{counter[0]}",
                                "opcode": "EventSemaphore",
                                "engine": ins["engine"],
                                "ins": [], "outs": [],
                                "debug": ins.get("debug", 0),
                                "sync_info": {"on_update": [], "on_wait": excess[i:i + 2]},
                            })
                    out.append(ins)
                blk["instructions"] = out
            if blk.get("blocks"):
                _fix_blocks(blk["blocks"], counter)

    orig = bass.Bass.to_json_bytes

    def to_json_bytes(self, *a, **kw):
        data = orig(self, *a, **kw)
        d = orjson.loads(data)
        counter = [0]
        for fn in d.get("functions", []):
            _fix_blocks(fn.get("blocks", []), counter)
        return orjson.dumps(d) if counter[0] else data

    bass.Bass.to_json_bytes = to_json_bytes
    bass.Bass._waitsplit_patched = True


# ---------------------------------------------------------------------------
# Host-side planning: per-roi corner grids, core/slot assignment, set layout
# ---------------------------------------------------------------------------
def _roi_meta(rois):
    """Per-roi level/batch + unique corner coords and per-sample corner ranks
    and bilinear weights."""
    scale_wh = np.sqrt((rois[:, 3] - rois[:, 1]) * (rois[:, 4] - rois[:, 2]))
    with np.errstate(divide="ignore"):
        tl = np.clip(np.floor(np.log2(scale_wh / FINEST + 1e-6)), 0, NLEV - 1)
    tl = (tl + 1e-5).astype(np.int32)
    g = np.arange(OUT, dtype=np.float64)[:, None] + (np.arange(SR, dtype=np.float64)[None, :] + 0.5) / SR
    metas = []
    for n in range(rois.shape[0]):
        l = int(tl[n])
        B, C_, H, W = FEAT_SHAPES[l]
        sc = 1.0 / STRIDES[l]
        x1 = rois[n, 1] * sc - 0.5
        y1 = rois[n, 2] * sc - 0.5
        rw = rois[n, 3] * sc - 0.5 - x1
        rh = rois[n, 4] * sc - 0.5 - y1
        y = y1 + (rh / OUT) * g  # [OUT, SR]
        x = x1 + (rw / OUT) * g
        vy = (y > -1) & (y < H)
        vx = (x > -1) & (x < W)
        yc = np.clip(y, 0.0, H - 1)
        xc = np.clip(x, 0.0, W - 1)
        y0 = np.minimum(np.floor(yc).astype(np.int64), H - 1)
        x0 = np.minimum(np.floor(xc).astype(np.int64), W - 1)
        y1i = np.minimum(y0 + 1, H - 1)
        x1i = np.minimum(x0 + 1, W - 1)
        ly = (yc - y0) * vy
        lx = (xc - x0) * vx
        hy = (1.0 - (yc - y0)) * vy
        hx = (1.0 - (xc - x0)) * vx
        ys = np.unique(np.concatenate([y0.ravel(), y1i.ravel()]))
        xs = np.unique(np.concatenate([x0.ravel(), x1i.ravel()]))
        metas.append(dict(
            l=l, b=int(rois[n, 0]), ys=ys, xs=xs, nrows=len(ys) * len(xs),
            y0r=np.searchsorted(ys, y0), y1r=np.searchsorted(ys, y1i),
            x0r=np.searchsorted(xs, x0), x1r=np.searchsorted(xs, x1i),
            ly=ly, lx=lx, hy=hy, hx=hx,
        ))
    return metas


def _plan(metas):
    """Assign rois to (core, slot); fixed slot row-caps shared by all cores."""
    nrows = np.array([m["nrows"] for m in metas])
    order = np.argsort(-nrows, kind="stable")
    loads = np.zeros(N_CORES, np.int64)
    counts = np.zeros(N_CORES, np.int64)
    core_rois = [[] for _ in range(N_CORES)]
    for i in order:
        avail = np.where(counts < NSLOT)[0]
        c = avail[np.argmin(loads[avail])]
        core_rois[c].append(int(i))
        loads[c] += nrows[i]
        counts[c] += 1
    # each core's list is already sorted desc by rows (dealt in global order)
    caps = np.zeros(NSLOT, np.int64)
    for c in range(N_CORES):
        for j, r in enumerate(core_rois[c]):
            caps[j] = max(caps[j], nrows[r])
    off = np.zeros(NSLOT + 1, np.int64)
    off[1:] = np.cumsum(caps)
    T = int(off[-1])
    G = -(-T // 128)
    # sets: one matmul per (group, slot) pair, ordered by (g, j)
    per_g = [[] for _ in range(G)]
    for j in range(NSLOT):
        g0, g1 = off[j] // 128, (off[j + 1] - 1) // 128
        for g in range(g0, g1 + 1):
            per_g[g].append((j, g == g0, g == g1))
    sets = []
    setmap = {}
    for g in range(G):
        for (j, st, sp) in per_g[g]:
            setmap[(g, j)] = len(sets)
            sets.append((g, j, st, sp))
    return core_rois, off, G, sets, setmap


def _build_core_inputs(feats16, metas, core_list, off, G, sets, setmap):
    S = len(sets)
    stream = np.zeros((128, G, C), np.float16)
    wacc = np.zeros((S, 128, NBIN), np.float32)
    for j, r in enumerate(core_list):
        m = metas[r]
        ys, xs = m["ys"], m["xs"]
        nX = len(xs)
        fT = feats16[m["l"]][m["b"]]  # [H, W, C] channels-last fp16
        block = fT[np.ix_(ys, xs)].reshape(-1, C)
        rows = int(off[j]) + np.arange(block.shape[0])
        stream[rows % 128, rows // 128, :] = block
        # bilinear weights onto (corner-row, bin)
        bi = (np.arange(OUT)[:, None, None, None] * OUT
              + np.arange(OUT)[None, None, :, None])  # [7,1,7,1]
        bins = np.broadcast_to(bi, (OUT, SR, OUT, SR))
        y0r, y1r = m["y0r"], m["y1r"]  # [7,2]
        x0r, x1r = m["x0r"], m["x1r"]
        ly, hy = m["ly"], m["hy"]
        lx, hx = m["lx"], m["hx"]
        q = 1.0 / (SR * SR)
        rows4, w4 = [], []
        for yr, wy in ((y0r, hy), (y1r, ly)):
            for xr, wx in ((x0r, hx), (x1r, lx)):
                rr = yr[:, :, None, None] * nX + xr[None, None, :, :]
                ww = wy[:, :, None, None] * wx[None, None, :, :] * q
                rows4.append(np.broadcast_to(rr, bins.shape).ravel())
                w4.append(np.broadcast_to(ww, bins.shape).ravel())
        rowa = int(off[j]) + np.concatenate(rows4)
        wa = np.concatenate(w4)
        bina = np.tile(bins.ravel(), 4)
        ga = rowa // 128
        sa = np.array([setmap[(int(g), j)] for g in ga])
        np.add.at(wacc, (sa, rowa % 128, bina), wa)
    wts = np.ascontiguousarray(wacc.astype(np.float16).transpose(1, 0, 2)).reshape(128, S * NBIN)
    return stream.reshape(128, G * C), wts


# ---------------------------------------------------------------------------
# Bass program
# ---------------------------------------------------------------------------
def _build_program(G, S, sets):
    import concourse.bacc as bacc
    import concourse.mybir as mybir
    import concourse.tile as tile

    _install_bir_waitsplit()
    nc = bacc.Bacc("TRN2", debug=False, enable_asserts=True, num_devices=N_CORES)

    stream_d = nc.dram_tensor("stream", [128, G * C], mybir.dt.float16, kind="ExternalInput")
    w_d = nc.dram_tensor("wts", [128, S * NBIN], mybir.dt.float16, kind="ExternalInput")
    out_d = nc.dram_tensor("out", [NBIN, NSLOT * C], mybir.dt.float16, kind="ExternalOutput")

    nchunk = -(-G // CHUNK_G)
    chunk_bounds = [(cidx * CHUNK_G, min((cidx + 1) * CHUNK_G, G)) for cidx in range(nchunk)]
    per_g = [[] for _ in range(G)]
    for s, (g, j, st, sp) in enumerate(sets):
        per_g[g].append(s)

    with tile.TileContext(nc) as tc:
        with (
            tc.tile_pool(name="fp", bufs=4) as fp,
            tc.tile_pool(name="wp", bufs=1) as wp,
            tc.tile_pool(name="sp", bufs=4) as sp,
            tc.tile_pool(name="pp", bufs=8, space="PSUM") as pp,
        ):
            wt = wp.tile([128, S * NBIN], mybir.dt.float16, name="wt")
            nc.sync.dma_start(wt[:], w_d[:])
            st_tiles = [sp.tile([NBIN, 8 * C], mybir.dt.float16, tag=f"st{k}", name=f"st{k}")
                        for k in range(4)]
            ps_of = {}
            for cidx, (glo, ghi) in enumerate(chunk_bounds):
                ft = fp.tile([128, CHUNK_G * C], mybir.dt.float16, tag="ft", name=f"ft{cidx}")
                nc.sync.dma_start(ft[:, :(ghi - glo) * C], stream_d[:, glo * C:ghi * C])
                for g in range(glo, ghi):
                    for s in per_g[g]:
                        _, j, st, sp_ = sets[s]
                        t = j // 2
                        if st and j % 2 == 0:
                            ps_of[t] = pp.tile([NBIN, 2 * C], mybir.dt.float32, tag="ps", name=f"ps_{t}")
                        ps = ps_of[t]
                        half = (j % 2) * C
                        nc.tensor.matmul(
                            out=ps[:, half:half + C],
                            lhsT=wt[:, s * NBIN:(s + 1) * NBIN],
                            rhs=ft[:, (g - glo) * C:(g - glo + 1) * C],
                            start=st, stop=sp_,
                        )
                        if sp_ and j % 2 == 1:
                            k, q = t // 4, t % 4
                            dst = st_tiles[k][:, q * 2 * C:(q + 1) * 2 * C]
                            nc.vector.tensor_copy(dst, ps[:, :])
                            if q == 3:
                                nc.sync.dma_start(
                                    out_d[:, k * 8 * C:(k + 1) * 8 * C], st_tiles[k][:])
    nc.compile()
    return nc


# ---------------------------------------------------------------------------
# Entry points
# ---------------------------------------------------------------------------
def _prepare(feat0, feat1, feat2, feat3, rois):
    feats = [np.asarray(f, np.float32) for f in (feat0, feat1, feat2, feat3)]
    rois = np.asarray(rois, np.float32)
    feats16 = [f.transpose(0, 2, 3, 1).astype(np.float16) for f in feats]
    metas = _roi_meta(rois)
    core_rois, off, G, sets, setmap = _plan(metas)
    in_maps = []
    for c in range(N_CORES):
        stream, wts = _build_core_inputs(feats16, metas, core_rois[c], off, G, sets, setmap)
        in_maps.append({"stream": stream, "wts": wts})
    return core_rois, G, sets, in_maps


def _unpack(out_np, core_list):
    """[49, NSLOT*C] fp16 device output -> dict roi_id -> [C, 7, 7] fp32."""
    res = {}
    for j in range(NSLOT):
        v = out_np[:, j * C:(j + 1) * C].astype(np.float32)  # [49, C]
        res[core_list[j]] = np.ascontiguousarray(v.T).reshape(C, OUT, OUT)
    return res


def kernel(feat0, feat1, feat2, feat3, rois):
    from concourse.bass_utils import run_bass_kernel_spmd

    core_rois, G, sets, in_maps = _prepare(feat0, feat1, feat2, feat3, rois)
    nc = _build_program(G, len(sets), sets)
    res = run_bass_kernel_spmd(nc, in_maps, core_ids=list(range(N_CORES)), trace=False)
    out = np.zeros((N_ROIS, C, OUT, OUT), np.float32)
    for c in range(N_CORES):
        for rid, v in _unpack(res.results[c]["out"], core_rois[c]).items():
            out[rid] = v
    return out


# Testing hook: emulate the device math in numpy (same stream/wts data).
def emulate(feat0, feat1, feat2, feat3, rois):
    core_rois, G, sets, in_maps = _prepare(feat0, feat1, feat2, feat3, rois)
    out = np.zeros((N_ROIS, C, OUT, OUT), np.float32)
    for c in range(N_CORES):
        stream = in_maps[c]["stream"].reshape(128, G, C)
        wts = in_maps[c]["wts"].reshape(128, len(sets), NBIN)
        acc = {}
        for s, (g, j, st, sp_) in enumerate(sets):
            part = wts[:, s, :].astype(np.float32).T @ stream[:, g, :].astype(np.float32)
            if st:
                acc[j] = part
            else:
                acc[j] += part
        dev = np.zeros((NBIN, NSLOT * C), np.float16)
        for j in range(NSLOT):
            dev[:, j * C:(j + 1) * C] = acc[j].astype(np.float16)
        for rid, v in _unpack(dev, core_rois[c]).items():
            out[rid] = v
    return out


# revision 6
# speedup vs baseline: 2.3584x; 1.0555x over previous
"""Multi-level (FPN) DeformRoIPool (zero-offset == aligned RoIAlign) for Trainium2.

Strategy (8 NeuronCores, SPMD, one Bass program):
- Host computes, per ROI, the unique corner positions (y, x) its 196 bilinear
  samples touch and packs those feature rows (C=256, fp16) into a contiguous
  per-core stream; duplicate corners are fetched once (~2.7x dedup vs per-
  sample gathers). ROIs are LPT-balanced across cores into 32 fixed "slots"
  with per-slot row capacities shared by all cores, so one compiled program
  serves all 8 cores.
- Device does plain chunked dma_starts (no gather): stream chunks + matching
  weight blocks double/triple-buffered, then one [K=128, 49] x [K=128, C]
  matmul per (group, slot) accumulating bins in PSUM (fp32). Two slots share
  one PSUM bank [49, 512]; DVE drains pairs to an fp16 staging tile; 4 batched
  stores write [98, 1024] blocks out.
- Host unpacks [slot, bin, C] -> [N, C, 7, 7].
"""
import numpy as np

OUT = 7
SR = 2
STRIDES = (4, 8, 16, 32)
FINEST = 56.0
NLEV = 4
C = 256
N_ROIS = 256
N_CORES = 8
NSLOT = N_ROIS // N_CORES           # 32 roi slots per core
NBIN = OUT * OUT                    # 49
CHUNK_SCHED = (4, 6, 8, 10)         # leading chunk sizes (groups); then steady size
CHUNK_STEADY = 12                   # steady-state groups per DMA chunk
FEAT_SHAPES = [(2, 256, 200, 200), (2, 256, 100, 100), (2, 256, 50, 50), (2, 256, 25, 25)]


# ---------------------------------------------------------------------------
# BIR fix: this container's walrus rejects >1 embedded sem wait per
# instruction (2 on EventSemaphore). Split excess waits onto EventSemaphore
# carriers at serialization time.
# ---------------------------------------------------------------------------
def _install_bir_waitsplit():
    import orjson
    import concourse.bass as bass

    if getattr(bass.Bass, "_waitsplit_patched", False):
        return

    def _fix_blocks(blocks, counter):
        for blk in blocks:
            insts = blk.get("instructions")
            if insts:
                out = []
                for ins in insts:
                    si = ins.get("sync_info")
                    ow = (si or {}).get("on_wait") or []
                    limit = 2 if ins.get("opcode") == "EventSemaphore" else 1
                    if len(ow) > limit:
                        excess = ow[: len(ow) - limit]
                        si["on_wait"] = ow[len(ow) - limit:]
                        for i in range(0, len(excess), 2):
                            counter[0] += 1
                            out.append({
                                "name": f"I-waitsplit-{counter[0]}",
                                "opcode": "EventSemaphore",
                                "engine": ins["engine"],
                                "ins": [], "outs": [],
                                "debug": ins.get("debug", 0),
                                "sync_info": {"on_update": [], "on_wait": excess[i:i + 2]},
                            })
                    out.append(ins)
                blk["instructions"] = out
            if blk.get("blocks"):
                _fix_blocks(blk["blocks"], counter)

    orig = bass.Bass.to_json_bytes

    def to_json_bytes(self, *a, **kw):
        data = orig(self, *a, **kw)
        d = orjson.loads(data)
        counter = [0]
        for fn in d.get("functions", []):
            _fix_blocks(fn.get("blocks", []), counter)
        return orjson.dumps(d) if counter[0] else data

    bass.Bass.to_json_bytes = to_json_bytes
    bass.Bass._waitsplit_patched = True


# ---------------------------------------------------------------------------
# Host-side planning: per-roi corner grids, core/slot assignment, set layout
# ---------------------------------------------------------------------------
def _roi_meta(rois):
    """Per-roi level/batch + unique corner coords and per-sample corner ranks
    and bilinear weights."""
    scale_wh = np.sqrt((rois[:, 3] - rois[:, 1]) * (rois[:, 4] - rois[:, 2]))
    with np.errstate(divide="ignore"):
        tl = np.clip(np.floor(np.log2(scale_wh / FINEST + 1e-6)), 0, NLEV - 1)
    tl = (tl + 1e-5).astype(np.int32)
    g = np.arange(OUT, dtype=np.float64)[:, None] + (np.arange(SR, dtype=np.float64)[None, :] + 0.5) / SR
    metas = []
    for n in range(rois.shape[0]):
        l = int(tl[n])
        B, C_, H, W = FEAT_SHAPES[l]
        sc = 1.0 / STRIDES[l]
        x1 = rois[n, 1] * sc - 0.5
        y1 = rois[n, 2] * sc - 0.5
        rw = rois[n, 3] * sc - 0.5 - x1
        rh = rois[n, 4] * sc - 0.5 - y1
        y = y1 + (rh / OUT) * g  # [OUT, SR]
        x = x1 + (rw / OUT) * g
        vy = (y > -1) & (y < H)
        vx = (x > -1) & (x < W)
        yc = np.clip(y, 0.0, H - 1)
        xc = np.clip(x, 0.0, W - 1)
        y0 = np.minimum(np.floor(yc).astype(np.int64), H - 1)
        x0 = np.minimum(np.floor(xc).astype(np.int64), W - 1)
        y1i = np.minimum(y0 + 1, H - 1)
        x1i = np.minimum(x0 + 1, W - 1)
        ly = (yc - y0) * vy
        lx = (xc - x0) * vx
        hy = (1.0 - (yc - y0)) * vy
        hx = (1.0 - (xc - x0)) * vx
        ys = np.unique(np.concatenate([y0.ravel(), y1i.ravel()]))
        xs = np.unique(np.concatenate([x0.ravel(), x1i.ravel()]))
        metas.append(dict(
            l=l, b=int(rois[n, 0]), ys=ys, xs=xs, nrows=len(ys) * len(xs),
            y0r=np.searchsorted(ys, y0), y1r=np.searchsorted(ys, y1i),
            x0r=np.searchsorted(xs, x0), x1r=np.searchsorted(xs, x1i),
            ly=ly, lx=lx, hy=hy, hx=hx,
        ))
    return metas


def _plan(metas):
    """Assign rois to (core, slot); fixed slot row-caps shared by all cores."""
    nrows = np.array([m["nrows"] for m in metas])
    order = np.argsort(-nrows, kind="stable")
    loads = np.zeros(N_CORES, np.int64)
    counts = np.zeros(N_CORES, np.int64)
    core_rois = [[] for _ in range(N_CORES)]
    for i in order:
        avail = np.where(counts < NSLOT)[0]
        c = avail[np.argmin(loads[avail])]
        core_rois[c].append(int(i))
        loads[c] += nrows[i]
        counts[c] += 1
    # each core's list is already sorted desc by rows (dealt in global order)
    caps = np.zeros(NSLOT, np.int64)
    for c in range(N_CORES):
        for j, r in enumerate(core_rois[c]):
            caps[j] = max(caps[j], nrows[r])
    off = np.zeros(NSLOT + 1, np.int64)
    off[1:] = np.cumsum(caps)
    T = int(off[-1])
    G = -(-T // 128)
    # sets: one matmul per (group, slot) pair, ordered by (g, j)
    per_g = [[] for _ in range(G)]
    for j in range(NSLOT):
        g0, g1 = off[j] // 128, (off[j + 1] - 1) // 128
        for g in range(g0, g1 + 1):
            per_g[g].append((j, g == g0, g == g1))
    sets = []
    setmap = {}
    for g in range(G):
        for (j, st, sp) in per_g[g]:
            setmap[(g, j)] = len(sets)
            sets.append((g, j, st, sp))
    return core_rois, off, G, sets, setmap


def _build_core_inputs(feats16, metas, core_list, off, G, sets, setmap):
    S = len(sets)
    stream = np.zeros((128, G, C), np.float16)
    wacc = np.zeros((S, 128, NBIN), np.float32)
    for j, r in enumerate(core_list):
        m = metas[r]
        ys, xs = m["ys"], m["xs"]
        nX = len(xs)
        fT = feats16[m["l"]][m["b"]]  # [H, W, C] channels-last fp16
        block = fT[np.ix_(ys, xs)].reshape(-1, C)
        rows = int(off[j]) + np.arange(block.shape[0])
        stream[rows % 128, rows // 128, :] = block
        # bilinear weights onto (corner-row, bin)
        bi = (np.arange(OUT)[:, None, None, None] * OUT
              + np.arange(OUT)[None, None, :, None])  # [7,1,7,1]
        bins = np.broadcast_to(bi, (OUT, SR, OUT, SR))
        y0r, y1r = m["y0r"], m["y1r"]  # [7,2]
        x0r, x1r = m["x0r"], m["x1r"]
        ly, hy = m["ly"], m["hy"]
        lx, hx = m["lx"], m["hx"]
        q = 1.0 / (SR * SR)
        rows4, w4 = [], []
        for yr, wy in ((y0r, hy), (y1r, ly)):
            for xr, wx in ((x0r, hx), (x1r, lx)):
                rr = yr[:, :, None, None] * nX + xr[None, None, :, :]
                ww = wy[:, :, None, None] * wx[None, None, :, :] * q
                rows4.append(np.broadcast_to(rr, bins.shape).ravel())
                w4.append(np.broadcast_to(ww, bins.shape).ravel())
        rowa = int(off[j]) + np.concatenate(rows4)
        wa = np.concatenate(w4)
        bina = np.tile(bins.ravel(), 4)
        ga = rowa // 128
        sa = np.array([setmap[(int(g), j)] for g in ga])
        np.add.at(wacc, (sa, rowa % 128, bina), wa)
    wts = np.ascontiguousarray(wacc.astype(np.float16).transpose(1, 0, 2)).reshape(128, S * NBIN)
    return stream.reshape(128, G * C), wts


# ---------------------------------------------------------------------------
# Bass program
# ---------------------------------------------------------------------------
def _build_program(G, S, sets):
    import concourse.bacc as bacc
    import concourse.mybir as mybir
    import concourse.tile as tile

    _install_bir_waitsplit()
    nc = bacc.Bacc("TRN2", debug=False, enable_asserts=True, num_devices=N_CORES)

    stream_d = nc.dram_tensor("stream", [128, G * C], mybir.dt.float16, kind="ExternalInput")
    w_d = nc.dram_tensor("wts", [128, S * NBIN], mybir.dt.float16, kind="ExternalInput")
    out_d = nc.dram_tensor("out", [NBIN, NSLOT * C], mybir.dt.float16, kind="ExternalOutput")

    chunk_bounds = []
    glo = 0
    cidx = 0
    while glo < G:
        sz = CHUNK_SCHED[cidx] if cidx < len(CHUNK_SCHED) else CHUNK_STEADY
        chunk_bounds.append((glo, min(glo + sz, G)))
        glo += sz
        cidx += 1
    per_g = [[] for _ in range(G)]
    for s, (g, j, st, sp) in enumerate(sets):
        per_g[g].append(s)
    # split weights: wt0 covers sets of the first 2 chunks, wt1 the rest
    g_split = chunk_bounds[1][1] if len(chunk_bounds) > 1 else G
    s_split = next((s for s, (g, _, _, _) in enumerate(sets) if g >= g_split), S)

    with tile.TileContext(nc) as tc:
        with (
            tc.tile_pool(name="fp", bufs=5) as fp,
            tc.tile_pool(name="wp", bufs=1) as wp,
            tc.tile_pool(name="sp", bufs=8) as sp,
            tc.tile_pool(name="pp", bufs=8, space="PSUM") as pp,
        ):
            wt0 = wp.tile([128, max(s_split, 1) * NBIN], mybir.dt.float16, name="wt0")
            nc.sync.dma_start(wt0[:], w_d[:, :s_split * NBIN])
            wt1 = None
            if s_split < S:
                wt1 = wp.tile([128, (S - s_split) * NBIN], mybir.dt.float16, name="wt1")
                nc.sync.dma_start(wt1[:], w_d[:, s_split * NBIN:])
            st_tiles = [sp.tile([NBIN, 4 * C], mybir.dt.float16, tag=f"st{k}", name=f"st{k}")
                        for k in range(8)]
            ps_of = {}
            for cidx, (glo, ghi) in enumerate(chunk_bounds):
                ft = fp.tile([128, CHUNK_STEADY * C], mybir.dt.float16, tag="ft", name=f"ft{cidx}")
                nc.sync.dma_start(ft[:, :(ghi - glo) * C], stream_d[:, glo * C:ghi * C])
                for g in range(glo, ghi):
                    for s in per_g[g]:
                        _, j, st, sp_ = sets[s]
                        t = j // 2
                        if st and j % 2 == 0:
                            ps_of[t] = pp.tile([NBIN, 2 * C], mybir.dt.float32, tag="ps", name=f"ps_{t}")
                        ps = ps_of[t]
                        half = (j % 2) * C
                        if s < s_split:
                            lhsT = wt0[:, s * NBIN:(s + 1) * NBIN]
                        else:
                            lhsT = wt1[:, (s - s_split) * NBIN:(s - s_split + 1) * NBIN]
                        nc.tensor.matmul(
                            out=ps[:, half:half + C],
                            lhsT=lhsT,
                            rhs=ft[:, (g - glo) * C:(g - glo + 1) * C],
                            start=st, stop=sp_,
                        )
                        if sp_ and j % 2 == 1:
                            k, q = t // 2, t % 2
                            dst = st_tiles[k][:, q * 2 * C:(q + 1) * 2 * C]
                            nc.vector.tensor_copy(dst, ps[:, :])
                            if q == 1:
                                nc.sync.dma_start(
                                    out_d[:, k * 4 * C:(k + 1) * 4 * C], st_tiles[k][:])
    nc.compile()
    return nc


# ---------------------------------------------------------------------------
# Entry points
# ---------------------------------------------------------------------------
def _prepare(feat0, feat1, feat2, feat3, rois):
    feats = [np.asarray(f, np.float32) for f in (feat0, feat1, feat2, feat3)]
    rois = np.asarray(rois, np.float32)
    feats16 = [f.transpose(0, 2, 3, 1).astype(np.float16) for f in feats]
    metas = _roi_meta(rois)
    core_rois, off, G, sets, setmap = _plan(metas)
    in_maps = []
    for c in range(N_CORES):
        stream, wts = _build_core_inputs(feats16, metas, core_rois[c], off, G, sets, setmap)
        in_maps.append({"stream": stream, "wts": wts})
    return core_rois, G, sets, in_maps


def _unpack(out_np, core_list):
    """[49, NSLOT*C] fp16 device output -> dict roi_id -> [C, 7, 7] fp32."""
    res = {}
    for j in range(NSLOT):
        v = out_np[:, j * C:(j + 1) * C].astype(np.float32)  # [49, C]
        res[core_list[j]] = np.ascontiguousarray(v.T).reshape(C, OUT, OUT)
    return res


def kernel(feat0, feat1, feat2, feat3, rois):
    from concourse.bass_utils import run_bass_kernel_spmd

    core_rois, G, sets, in_maps = _prepare(feat0, feat1, feat2, feat3, rois)
    nc = _build_program(G, len(sets), sets)
    res = run_bass_kernel_spmd(nc, in_maps, core_ids=list(range(N_CORES)), trace=False)
    out = np.zeros((N_ROIS, C, OUT, OUT), np.float32)
    for c in range(N_CORES):
        for rid, v in _unpack(res.results[c]["out"], core_rois[c]).items():
            out[rid] = v
    return out


# Testing hook: emulate the device math in numpy (same stream/wts data).
def emulate(feat0, feat1, feat2, feat3, rois):
    core_rois, G, sets, in_maps = _prepare(feat0, feat1, feat2, feat3, rois)
    out = np.zeros((N_ROIS, C, OUT, OUT), np.float32)
    for c in range(N_CORES):
        stream = in_maps[c]["stream"].reshape(128, G, C)
        wts = in_maps[c]["wts"].reshape(128, len(sets), NBIN)
        acc = {}
        for s, (g, j, st, sp_) in enumerate(sets):
            part = wts[:, s, :].astype(np.float32).T @ stream[:, g, :].astype(np.float32)
            if st:
                acc[j] = part
            else:
                acc[j] += part
        dev = np.zeros((NBIN, NSLOT * C), np.float16)
        for j in range(NSLOT):
            dev[:, j * C:(j + 1) * C] = acc[j].astype(np.float16)
        for rid, v in _unpack(dev, core_rois[c]).items():
            out[rid] = v
    return out


# revision 7
# speedup vs baseline: 2.4574x; 1.0420x over previous
"""Multi-level (FPN) DeformRoIPool (zero-offset == aligned RoIAlign) for Trainium2.

Strategy (8 NeuronCores, SPMD, one Bass program):
- Host computes, per ROI, the unique corner positions (y, x) its 196 bilinear
  samples touch and packs those feature rows (C=256, fp16) into a contiguous
  per-core stream; duplicate corners are fetched once (~2.7x dedup vs per-
  sample gathers). ROIs are LPT-balanced across cores into 32 fixed "slots"
  with per-slot row capacities shared by all cores, so one compiled program
  serves all 8 cores.
- Device does plain chunked dma_starts (no gather): stream chunks + matching
  weight blocks double/triple-buffered, then one [K=128, 49] x [K=128, C]
  matmul per (group, slot) accumulating bins in PSUM (fp32). Two slots share
  one PSUM bank [49, 512]; DVE drains pairs to an fp16 staging tile; 4 batched
  stores write [98, 1024] blocks out.
- Host unpacks [slot, bin, C] -> [N, C, 7, 7].
"""
import numpy as np

OUT = 7
SR = 2
STRIDES = (4, 8, 16, 32)
FINEST = 56.0
NLEV = 4
C = 256
N_ROIS = 256
N_CORES = 8
NSLOT = N_ROIS // N_CORES           # 32 roi slots per core
NBIN = OUT * OUT                    # 49
CHUNK_SCHED = (4, 6, 8, 10)         # leading chunk sizes (groups); then steady size
CHUNK_STEADY = 12                   # steady-state groups per DMA chunk
FEAT_SHAPES = [(2, 256, 200, 200), (2, 256, 100, 100), (2, 256, 50, 50), (2, 256, 25, 25)]


# ---------------------------------------------------------------------------
# BIR fix: this container's walrus rejects >1 embedded sem wait per
# instruction (2 on EventSemaphore). Split excess waits onto EventSemaphore
# carriers at serialization time.
# ---------------------------------------------------------------------------
def _install_bir_waitsplit():
    import orjson
    import concourse.bass as bass

    if getattr(bass.Bass, "_waitsplit_patched", False):
        return

    def _fix_blocks(blocks, counter):
        for blk in blocks:
            insts = blk.get("instructions")
            if insts:
                out = []
                for ins in insts:
                    si = ins.get("sync_info")
                    ow = (si or {}).get("on_wait") or []
                    limit = 2 if ins.get("opcode") == "EventSemaphore" else 1
                    if len(ow) > limit:
                        excess = ow[: len(ow) - limit]
                        si["on_wait"] = ow[len(ow) - limit:]
                        for i in range(0, len(excess), 2):
                            counter[0] += 1
                            out.append({
                                "name": f"I-waitsplit-{counter[0]}",
                                "opcode": "EventSemaphore",
                                "engine": ins["engine"],
                                "ins": [], "outs": [],
                                "debug": ins.get("debug", 0),
                                "sync_info": {"on_update": [], "on_wait": excess[i:i + 2]},
                            })
                    out.append(ins)
                blk["instructions"] = out
            if blk.get("blocks"):
                _fix_blocks(blk["blocks"], counter)

    orig = bass.Bass.to_json_bytes

    def to_json_bytes(self, *a, **kw):
        data = orig(self, *a, **kw)
        d = orjson.loads(data)
        counter = [0]
        for fn in d.get("functions", []):
            _fix_blocks(fn.get("blocks", []), counter)
        return orjson.dumps(d) if counter[0] else data

    bass.Bass.to_json_bytes = to_json_bytes
    bass.Bass._waitsplit_patched = True


# ---------------------------------------------------------------------------
# Host-side planning: per-roi corner grids, core/slot assignment, set layout
# ---------------------------------------------------------------------------
def _roi_meta(rois):
    """Per-roi level/batch + unique corner coords and per-sample corner ranks
    and bilinear weights."""
    scale_wh = np.sqrt((rois[:, 3] - rois[:, 1]) * (rois[:, 4] - rois[:, 2]))
    with np.errstate(divide="ignore"):
        tl = np.clip(np.floor(np.log2(scale_wh / FINEST + 1e-6)), 0, NLEV - 1)
    tl = (tl + 1e-5).astype(np.int32)
    g = np.arange(OUT, dtype=np.float64)[:, None] + (np.arange(SR, dtype=np.float64)[None, :] + 0.5) / SR
    metas = []
    for n in range(rois.shape[0]):
        l = int(tl[n])
        B, C_, H, W = FEAT_SHAPES[l]
        sc = 1.0 / STRIDES[l]
        x1 = rois[n, 1] * sc - 0.5
        y1 = rois[n, 2] * sc - 0.5
        rw = rois[n, 3] * sc - 0.5 - x1
        rh = rois[n, 4] * sc - 0.5 - y1
        y = y1 + (rh / OUT) * g  # [OUT, SR]
        x = x1 + (rw / OUT) * g
        vy = (y > -1) & (y < H)
        vx = (x > -1) & (x < W)
        yc = np.clip(y, 0.0, H - 1)
        xc = np.clip(x, 0.0, W - 1)
        y0 = np.minimum(np.floor(yc).astype(np.int64), H - 1)
        x0 = np.minimum(np.floor(xc).astype(np.int64), W - 1)
        y1i = np.minimum(y0 + 1, H - 1)
        x1i = np.minimum(x0 + 1, W - 1)
        ly = (yc - y0) * vy
        lx = (xc - x0) * vx
        hy = (1.0 - (yc - y0)) * vy
        hx = (1.0 - (xc - x0)) * vx
        ys = np.unique(np.concatenate([y0.ravel(), y1i.ravel()]))
        xs = np.unique(np.concatenate([x0.ravel(), x1i.ravel()]))
        metas.append(dict(
            l=l, b=int(rois[n, 0]), ys=ys, xs=xs, nrows=len(ys) * len(xs),
            y0r=np.searchsorted(ys, y0), y1r=np.searchsorted(ys, y1i),
            x0r=np.searchsorted(xs, x0), x1r=np.searchsorted(xs, x1i),
            ly=ly, lx=lx, hy=hy, hx=hx,
        ))
    return metas


def _plan(metas):
    """Assign rois to (core, slot); fixed slot row-caps shared by all cores."""
    nrows = np.array([m["nrows"] for m in metas])
    order = np.argsort(-nrows, kind="stable")
    loads = np.zeros(N_CORES, np.int64)
    counts = np.zeros(N_CORES, np.int64)
    core_rois = [[] for _ in range(N_CORES)]
    for i in order:
        avail = np.where(counts < NSLOT)[0]
        c = avail[np.argmin(loads[avail])]
        core_rois[c].append(int(i))
        loads[c] += nrows[i]
        counts[c] += 1
    # each core's list is already sorted desc by rows (dealt in global order)
    caps = np.zeros(NSLOT, np.int64)
    for c in range(N_CORES):
        for j, r in enumerate(core_rois[c]):
            caps[j] = max(caps[j], nrows[r])
    off = np.zeros(NSLOT + 1, np.int64)
    off[1:] = np.cumsum(caps)
    T = int(off[-1])
    G = -(-T // 128)
    # sets: one matmul per (group, slot) pair, ordered by (g, j)
    per_g = [[] for _ in range(G)]
    for j in range(NSLOT):
        g0, g1 = off[j] // 128, (off[j + 1] - 1) // 128
        for g in range(g0, g1 + 1):
            per_g[g].append((j, g == g0, g == g1))
    sets = []
    setmap = {}
    for g in range(G):
        for (j, st, sp) in per_g[g]:
            setmap[(g, j)] = len(sets)
            sets.append((g, j, st, sp))
    return core_rois, off, G, sets, setmap


def _chunk_plan(G, sets):
    """Chunk bounds + per-chunk set ranges + fused column offsets."""
    chunk_bounds = []
    glo = 0
    cidx = 0
    while glo < G:
        sz = CHUNK_SCHED[cidx] if cidx < len(CHUNK_SCHED) else CHUNK_STEADY
        chunk_bounds.append((glo, min(glo + sz, G)))
        glo += sz
        cidx += 1
    chunks = []
    coff = 0
    for (glo, ghi) in chunk_bounds:
        s_in = [s for s, (g, j, st, sp) in enumerate(sets) if glo <= g < ghi]
        slo, shi = s_in[0], s_in[-1] + 1
        width = (ghi - glo) * C + (shi - slo) * NBIN
        chunks.append((glo, ghi, slo, shi, coff, width))
        coff += width
    return chunks, coff


def _build_core_inputs(feats16, metas, core_list, off, G, sets, setmap):
    S = len(sets)
    stream = np.zeros((128, G, C), np.float16)
    wacc = np.zeros((S, 128, NBIN), np.float32)
    for j, r in enumerate(core_list):
        m = metas[r]
        ys, xs = m["ys"], m["xs"]
        nX = len(xs)
        fT = feats16[m["l"]][m["b"]]  # [H, W, C] channels-last fp16
        block = fT[np.ix_(ys, xs)].reshape(-1, C)
        rows = int(off[j]) + np.arange(block.shape[0])
        stream[rows % 128, rows // 128, :] = block
        # bilinear weights onto (corner-row, bin)
        bi = (np.arange(OUT)[:, None, None, None] * OUT
              + np.arange(OUT)[None, None, :, None])  # [7,1,7,1]
        bins = np.broadcast_to(bi, (OUT, SR, OUT, SR))
        y0r, y1r = m["y0r"], m["y1r"]  # [7,2]
        x0r, x1r = m["x0r"], m["x1r"]
        ly, hy = m["ly"], m["hy"]
        lx, hx = m["lx"], m["hx"]
        q = 1.0 / (SR * SR)
        rows4, w4 = [], []
        for yr, wy in ((y0r, hy), (y1r, ly)):
            for xr, wx in ((x0r, hx), (x1r, lx)):
                rr = yr[:, :, None, None] * nX + xr[None, None, :, :]
                ww = wy[:, :, None, None] * wx[None, None, :, :] * q
                rows4.append(np.broadcast_to(rr, bins.shape).ravel())
                w4.append(np.broadcast_to(ww, bins.shape).ravel())
        rowa = int(off[j]) + np.concatenate(rows4)
        wa = np.concatenate(w4)
        bina = np.tile(bins.ravel(), 4)
        ga = rowa // 128
        sa = np.array([setmap[(int(g), j)] for g in ga])
        np.add.at(wacc, (sa, rowa % 128, bina), wa)
    wts = wacc.astype(np.float16).transpose(1, 0, 2)  # [128, S, 49]
    chunks, totw = _chunk_plan(G, sets)
    fused = np.zeros((128, totw), np.float16)
    for (glo, ghi, slo, shi, coff, width) in chunks:
        nf = (ghi - glo) * C
        fused[:, coff:coff + nf] = stream[:, glo:ghi].reshape(128, nf)
        fused[:, coff + nf:coff + width] = wts[:, slo:shi].reshape(128, -1)
    return fused


# ---------------------------------------------------------------------------
# Bass program
# ---------------------------------------------------------------------------
def _build_program(G, S, sets):
    import concourse.bacc as bacc
    import concourse.mybir as mybir
    import concourse.tile as tile

    _install_bir_waitsplit()
    nc = bacc.Bacc("TRN2", debug=False, enable_asserts=False, num_devices=N_CORES)

    chunks, totw = _chunk_plan(G, sets)
    maxw = max(w for *_, w in chunks)
    stream_d = nc.dram_tensor("stream", [128, totw], mybir.dt.float16, kind="ExternalInput")
    out_d = nc.dram_tensor("out", [NBIN, NSLOT * C], mybir.dt.float16, kind="ExternalOutput")

    per_g = [[] for _ in range(G)]
    for s, (g, j, st, sp) in enumerate(sets):
        per_g[g].append(s)

    with tile.TileContext(nc) as tc:
        with (
            tc.tile_pool(name="fp", bufs=4) as fp,
            tc.tile_pool(name="sp", bufs=1) as sp,
            tc.tile_pool(name="pp", bufs=8, space="PSUM") as pp,
        ):
            st_tiles = [sp.tile([NBIN, 4 * C], mybir.dt.float16, tag=f"st{k}", name=f"st{k}")
                        for k in range(8)]
            ps_of = {}
            for cidx, (glo, ghi, slo, shi, coff, width) in enumerate(chunks):
                nf = (ghi - glo) * C
                ft = fp.tile([128, maxw], mybir.dt.float16, tag="ft", name=f"ft{cidx}")
                nc.sync.dma_start(ft[:, :width], stream_d[:, coff:coff + width])
                for g in range(glo, ghi):
                    for s in per_g[g]:
                        _, j, st, sp_ = sets[s]
                        t = j // 2
                        if st and j % 2 == 0:
                            ps_of[t] = pp.tile([NBIN, 2 * C], mybir.dt.float32, tag="ps", name=f"ps_{t}")
                        ps = ps_of[t]
                        half = (j % 2) * C
                        wb = nf + (s - slo) * NBIN
                        nc.tensor.matmul(
                            out=ps[:, half:half + C],
                            lhsT=ft[:, wb:wb + NBIN],
                            rhs=ft[:, (g - glo) * C:(g - glo + 1) * C],
                            start=st, stop=sp_,
                        )
                        if sp_ and j % 2 == 1:
                            k, q = t // 2, t % 2
                            dst = st_tiles[k][:, q * 2 * C:(q + 1) * 2 * C]
                            nc.vector.tensor_copy(dst, ps[:, :])
                            if q == 1:
                                nc.scalar.dma_start(
                                    out_d[:, k * 4 * C:(k + 1) * 4 * C], st_tiles[k][:])
    nc.compile()
    return nc


# ---------------------------------------------------------------------------
# Entry points
# ---------------------------------------------------------------------------
def _prepare(feat0, feat1, feat2, feat3, rois):
    feats = [np.asarray(f, np.float32) for f in (feat0, feat1, feat2, feat3)]
    rois = np.asarray(rois, np.float32)
    feats16 = [f.transpose(0, 2, 3, 1).astype(np.float16) for f in feats]
    metas = _roi_meta(rois)
    core_rois, off, G, sets, setmap = _plan(metas)
    in_maps = []
    for c in range(N_CORES):
        fused = _build_core_inputs(feats16, metas, core_rois[c], off, G, sets, setmap)
        in_maps.append({"stream": fused})
    return core_rois, G, sets, in_maps


def _unpack(out_np, core_list):
    """[49, NSLOT*C] fp16 device output -> dict roi_id -> [C, 7, 7] fp32."""
    res = {}
    for j in range(NSLOT):
        v = out_np[:, j * C:(j + 1) * C].astype(np.float32)  # [49, C]
        res[core_list[j]] = np.ascontiguousarray(v.T).reshape(C, OUT, OUT)
    return res


def kernel(feat0, feat1, feat2, feat3, rois):
    from concourse.bass_utils import run_bass_kernel_spmd

    core_rois, G, sets, in_maps = _prepare(feat0, feat1, feat2, feat3, rois)
    nc = _build_program(G, len(sets), sets)
    res = run_bass_kernel_spmd(nc, in_maps, core_ids=list(range(N_CORES)), trace=False)
    out = np.zeros((N_ROIS, C, OUT, OUT), np.float32)
    for c in range(N_CORES):
        for rid, v in _unpack(res.results[c]["out"], core_rois[c]).items():
            out[rid] = v
    return out


# Testing hook: emulate the device math in numpy (same stream/wts data).
def emulate(feat0, feat1, feat2, feat3, rois):
    core_rois, G, sets, in_maps = _prepare(feat0, feat1, feat2, feat3, rois)
    chunks, totw = _chunk_plan(G, sets)
    chunk_of_g = {}
    for ch in chunks:
        for g in range(ch[0], ch[1]):
            chunk_of_g[g] = ch
    out = np.zeros((N_ROIS, C, OUT, OUT), np.float32)
    for c in range(N_CORES):
        fused = in_maps[c]["stream"]
        acc = {}
        for s, (g, j, st, sp_) in enumerate(sets):
            glo, ghi, slo, shi, coff, width = chunk_of_g[g]
            nf = (ghi - glo) * C
            rhs = fused[:, coff + (g - glo) * C: coff + (g - glo + 1) * C].astype(np.float32)
            lhsT = fused[:, coff + nf + (s - slo) * NBIN: coff + nf + (s - slo + 1) * NBIN].astype(np.float32)
            part = lhsT.T @ rhs
            if st:
                acc[j] = part
            else:
                acc[j] += part
        dev = np.zeros((NBIN, NSLOT * C), np.float16)
        for j in range(NSLOT):
            dev[:, j * C:(j + 1) * C] = acc[j].astype(np.float16)
        for rid, v in _unpack(dev, core_rois[c]).items():
            out[rid] = v
    return out
